# revision 5
# baseline (speedup 1.0000x reference)
"""FLT (FAVOR+ linear attention with RFF positional features) Trainium2 kernel.

Sharding: 8 cores; core c handles batch b = c//2 and head-group g = c%2
(heads 4g..4g+3). Each core computes the partial output
sum_{h in group} (per-head attention @ W_h^T) as [N, 64]; host sums the
two groups per batch and adds the output bias.

Math (q-side eps dropped; validated 2.1e-3 vs reference, tol 2e-2):
  dash = [phi, x] @ PW_h^T           (phi = [cos(u),sin(u)], u from coords)
  K side: kr = exp(dash_k - dg_k) with dg_k folded in as activation bias,
  mask folded into vext; ctx accumulated as
    ctxT[65,257] = sum_n mask*[v|1] (outer) [exp(dash_k[n,:]-dg_k) | 1]
  col 256 gives [Mv|cnt] for the k-side eps fix-up:
    ctx = ratio*exp(-m_k)*ctxT[:, :256] + ratio*eps*ctxT[:, 256]
  Q side: qraw = exp(dash_q) raw; per-token rowmax rmax = max_f qraw
  (exact, via elementwise-max of the two 128-blocks then PE transpose):
    num[n,e] = (qraw @ ctx2)[n,e],   ctx2 = ctx @ W_h^T
    den[n]   = (qraw @ ksum)[n] + (NORM_EPS/ratio)*exp(dg_q[n])*rmax[n]
    out[n,e] = num/den
"""

import math
import os
from contextlib import ExitStack

import numpy as np
import ml_dtypes

import concourse.bass as bass
import concourse.bacc as bacc
import concourse.mybir as mybir
import concourse.tile as tile

BF16 = ml_dtypes.bfloat16

H, DH = 8, 64
SOFTMAX_TEMP = 1.0 / math.sqrt(DH)
SOFTMAX_EPS = 1e-6
NORM_EPS = 1e-6
B, N_FULL = 4, 8192
RATIO = 256 ** -0.5
MAGIC = float(1.5 * 2 ** 23)
INV2PI = float(1.0 / (2 * math.pi))

N_TOK = int(os.environ.get("FLT_NTOK", N_FULL))

_CACHED = {}


def _bf(x):
    return np.ascontiguousarray(x).astype(BF16)


def host_prep(query, key, value, coords, mask, w_rpe_weight, omega_dR,
              omega_dAngle, projection_matrix, out_w, out_b, n_tok=N_TOK):
    """Build the 8 per-core input maps (numpy only)."""
    w2 = w_rpe_weight.reshape(H, DH, 2, 2, 4).transpose(3, 0, 1, 2, 4)
    e = np.sum(np.exp(np.minimum(np.sum(w2, axis=2), 50.0)), axis=-1)  # c h r
    alpha, qw = e[0], e[1]
    new_qw = np.concatenate([qw[:, :1], qw], axis=-1)  # [H,3]
    sqrt_qw = np.sqrt(new_qw)

    P = projection_matrix  # [256, 192]
    sc = math.sqrt(2.0 / 64)
    # phi feature order here: [cosdR(32), cosdA(32), sindR(32), sindA(32)]
    # reference phi cols in P: cosdR->64+j, sindR->96+j, cosdA->128+j, sindA->160+j
    idx = np.concatenate([64 + np.arange(32), 128 + np.arange(32),
                          96 + np.arange(32), 160 + np.arange(32)])

    T = n_tok // 128
    in_maps = []
    for c in range(8):
        b = c // 2
        g = c % 2
        heads = [4 * g + i for i in range(4)]

        qs = query[b, :n_tok, 256 * g:256 * (g + 1)]
        ks = key[b, :n_tok, 256 * g:256 * (g + 1)]
        vs = value[b, :n_tok, 256 * g:256 * (g + 1)]

        qfm = _bf(qs.T.reshape(2, 128, n_tok))
        kfm = _bf(ks.T.reshape(2, 128, n_tok))
        qtm = _bf(qs)
        ktm = _bf(ks)

        vext = np.ones((n_tok, 4 * 65), np.float32)
        for h in range(4):
            vext[:, 65 * h:65 * h + 64] = vs[:, 64 * h:64 * h + 64]
        vext = _bf(vext)

        cfm = np.ones((4, n_tok), np.float32)
        cfm[:3] = coords[b, :n_tok, :].T
        cfm = _bf(cfm)  # [4, n], row 3 = 1.0

        omc = np.zeros((4, 512), np.float32)
        pwphi = np.zeros((128, 1024), np.float32)
        pwq = np.zeros((64, 1024), np.float32)
        ctxw = np.zeros((64, 256), np.float32)
        dgc8 = np.zeros((128, 8), np.float32)
        for h, hg in enumerate(heads):
            o = np.zeros((3, 64), np.float32)
            o[0, :32] = sqrt_qw[hg, 0] * omega_dR[0]
            o[1, :32] = sqrt_qw[hg, 1] * omega_dR[1]
            o[2, 32:] = sqrt_qw[hg, 2] * omega_dAngle[0]
            omc[:3, 128 * h:128 * h + 64] = o * INV2PI
            omc[:3, 128 * h + 64:128 * h + 128] = o * INV2PI
            omc[3, 128 * h:128 * h + 64] = 0.25   # +pi/2 (in turns) -> cos
            omc[3, 128 * h + 64:128 * h + 128] = 0.0
            a0 = math.sqrt(alpha[hg, 0]); a1 = math.sqrt(alpha[hg, 1])
            colscale = np.concatenate([np.full(32, a0), np.full(32, a1)] * 2) * sc
            pwphi[:, 256 * h:256 * h + 256] = (P[:, idx] * colscale[None, :]).T
            pwq[:, 256 * h:256 * h + 256] = (P[:, :64] * math.sqrt(SOFTMAX_TEMP)).T
            ctxw[:, 64 * h:64 * h + 64] = out_w[:, 64 * hg:64 * hg + 64].T
            dgc = np.float32(0.5 * (alpha[hg, 0] + alpha[hg, 1]))
            dgc8[:, h] = dgc
            dgc8[:, 4 + h] = -dgc

        mask_f = mask[b, :n_tok].astype(np.float32)
        mask_t = np.ascontiguousarray(mask_f.reshape(T, 128).T)  # [128, T]

        in_maps.append({
            "qfm": qfm, "kfm": kfm, "qtm": qtm, "ktm": ktm, "vext": vext,
            "cfm": cfm, "omc": _bf(omc),
            "pwphi": _bf(pwphi), "pwq": _bf(pwq),
            "ctxw": _bf(ctxw), "mask_t": mask_t,
            "dgc8": dgc8,
            "identb": _bf(np.eye(128, dtype=np.float32)),
            "identf": np.eye(128, dtype=np.float32),
        })
    return in_maps


def build_nc(n_tok=N_TOK):
    NT = n_tok
    T = NT // 128
    assert NT % 2048 == 0
    f32 = mybir.dt.float32
    bf16 = mybir.dt.bfloat16

    nc = bacc.Bacc()
    dp = nc.declare_dram_parameter
    qfm_d = dp("qfm", [2, 128, NT], bf16, isOutput=False)
    kfm_d = dp("kfm", [2, 128, NT], bf16, isOutput=False)
    qtm_d = dp("qtm", [NT, 256], bf16, isOutput=False)
    ktm_d = dp("ktm", [NT, 256], bf16, isOutput=False)
    vext_d = dp("vext", [NT, 260], bf16, isOutput=False)
    cfm_d = dp("cfm", [4, NT], bf16, isOutput=False)
    omc_d = dp("omc", [4, 512], bf16, isOutput=False)
    pwphi_d = dp("pwphi", [128, 1024], bf16, isOutput=False)
    pwq_d = dp("pwq", [64, 1024], bf16, isOutput=False)
    ctxw_d = dp("ctxw", [64, 256], bf16, isOutput=False)
    mask_t_d = dp("mask_t", [128, T], f32, isOutput=False)
    dgc8_d = dp("dgc8", [128, 8], f32, isOutput=False)
    identb_d = dp("identb", [128, 128], bf16, isOutput=False)
    identf_d = dp("identf", [128, 128], f32, isOutput=False)
    outp_d = dp("outp", [NT, 64], f32, isOutput=True)

    AX = mybir.AxisListType
    OP = mybir.AluOpType
    AF = mybir.ActivationFunctionType

    with tile.TileContext(nc) as tc, ExitStack() as ctx:
        consts = ctx.enter_context(tc.tile_pool(name="consts", bufs=1))
        persist = ctx.enter_context(tc.tile_pool(name="persist", bufs=1))
        pair_p = ctx.enter_context(tc.tile_pool(name="pair", bufs=1))
        phi_p = ctx.enter_context(tc.tile_pool(name="phip", bufs=1))
        qraw_p = ctx.enter_context(tc.tile_pool(name="qrawp", bufs=1))
        stream = ctx.enter_context(tc.tile_pool(name="stream", bufs=2))
        small = ctx.enter_context(tc.tile_pool(name="small", bufs=2))
        # PSUM budget (8 banks of 2KB/partition):
        #   ps_main [128,1024] f32 = 2 banks x bufs2 = 4
        #   ps_ctx  [65,257]       = 1 bank  x bufs1 = 1
        #   ps_aux  (transposes, fin assembly, pf) x bufs2 = 2
        ps_main = ctx.enter_context(tc.tile_pool(name="ps_main", bufs=2, space="PSUM"))
        ps_ctx = ctx.enter_context(tc.tile_pool(name="ps_ctx", bufs=1, space="PSUM"))
        ps_aux = ctx.enter_context(tc.tile_pool(name="ps_aux", bufs=2, space="PSUM"))

        # ---- constants to SBUF
        def load_const(name, shape, dt, src):
            t = consts.tile(shape, dt, tag=name, name=name)
            nc.sync.dma_start(t[:], src)
            return t

        cfm = load_const("cfm", [4, NT], bf16, cfm_d[:])
        omc = load_const("omc", [4, 512], bf16, omc_d[:])
        pwphi = load_const("pwphi", [128, 1024], bf16, pwphi_d[:])
        pwq = load_const("pwq", [64, 1024], bf16, pwq_d[:])
        ctxw = load_const("ctxw", [64, 256], bf16, ctxw_d[:])
        mask_t = load_const("mask_t", [128, T], f32, mask_t_d[:])
        dgc8 = load_const("dgc8", [128, 8], f32, dgc8_d[:])
        identb = load_const("identb", [128, 128], bf16, identb_d[:])
        identf = load_const("identf", [128, 128], f32, identf_d[:])
        onesb = consts.tile([128, 1], bf16)
        nc.vector.memset(onesb[:], 1.0)

        vextm = persist.tile([128, T, 260], bf16)
        nc.sync.dma_start(vextm[:], vext_d.rearrange("(t p) e -> p t e", p=128))

        dgq = persist.tile([128, 4, T], f32)
        dgk = persist.tile([128, 4, T], f32)
        acc = persist.tile([128, T * 64], f32)
        nc.vector.memset(acc[:], 0.0)

        # kr double-buffer pair with the 257th (ones) column preset once;
        # the ctx matmul picks up [Mv|cnt] from it for free.
        krA = persist.tile([128, 4, 257], bf16)
        krB = persist.tile([128, 4, 257], bf16)
        nc.vector.memset(krA[:, :, 256:257], 1.0)
        nc.vector.memset(krB[:, :, 256:257], 1.0)

        # mask folded into vext
        for t in range(T):
            nc.vector.tensor_scalar_mul(vextm[:, t, :], vextm[:, t, :],
                                        mask_t[:, t:t + 1])

        # ---- DG pass: per-token sum-of-squares of q (gpsimd) and k (vector)
        for t in range(T):
            qt = stream.tile([128, 256], bf16, tag="qtm")
            kt = stream.tile([128, 256], bf16, tag="ktm")
            nc.sync.dma_start(qt[:], qtm_d[128 * t:128 * (t + 1), :])
            nc.sync.dma_start(kt[:], ktm_d[128 * t:128 * (t + 1), :])
            for h in range(4):
                scrg = stream.tile([128, 64], bf16, tag="scrg")
                nc.scalar.activation(
                    scrg[:], qt[:, 64 * h:64 * h + 64], AF.Square,
                    accum_out=dgq[:, h, t:t + 1])
                scrv = stream.tile([128, 64], bf16, tag="scrv")
                nc.vector.scalar_tensor_tensor(
                    scrv[:], kt[:, 64 * h:64 * h + 64], 1.0,
                    kt[:, 64 * h:64 * h + 64], OP.mult, OP.mult,
                    accum_out=dgk[:, h, t:t + 1])

        # ---- per head
        for h in range(4):
            j, r0 = h // 2, 64 * (h % 2)
            qfm = pair_p.tile([64, NT], bf16, tag="qfm")
            kfm = pair_p.tile([64, NT], bf16, tag="kfm")
            nc.sync.dma_start(qfm[:], qfm_d[j, r0:r0 + 64, :])
            nc.sync.dma_start(kfm[:], kfm_d[j, r0:r0 + 64, :])

            # A. phi: u2 = omc^T coords; k = round(u_turns) via magic const;
            # red = u - k; phi = sin(2pi*red (+pi/2 bias folded into omc))
            phi = phi_p.tile([128, NT], bf16, tag="phi")
            for sl in range(NT // 1024):
                pu = ps_main.tile([128, 1024], f32, tag="main")
                for i in range(2):
                    cs = slice(1024 * sl + 512 * i, 1024 * sl + 512 * (i + 1))
                    nc.tensor.matmul(
                        pu[:, 512 * i:512 * (i + 1)],
                        omc[:, 128 * h:128 * (h + 1)],
                        cfm[:, cs], start=True, stop=True)
                khat = stream.tile([128, 1024], f32, tag="khat")
                nc.vector.tensor_scalar(khat[:], pu[:], MAGIC, MAGIC,
                                        OP.add, OP.subtract)
                red = stream.tile([128, 1024], f32, tag="red")
                nc.vector.scalar_tensor_tensor(red[:], khat[:], -1.0, pu[:],
                                               OP.mult, OP.add)
                nc.scalar.activation(phi[:, 1024 * sl:1024 * (sl + 1)], red[:],
                                     AF.Sin, bias=0.0, scale=float(2 * math.pi))

            # B. bias column for the K-side exp: -(0.5*temp*|k|^2 + dgc)
            negdgk = small.tile([128, T], f32, tag="negdgk")
            nc.vector.tensor_scalar(negdgk[:], dgk[:, h, :],
                                    float(-0.5 * SOFTMAX_TEMP),
                                    dgc8[:, 4 + h:5 + h], OP.mult, OP.add)

            # C. K pass: dash_k (token-major) -> exp(dash-dg) -> ctx
            mxk = small.tile([128, T], f32, tag="mxk")
            ctx_ps = ps_ctx.tile([65, 257], f32, tag="ctx")
            for sl in range(T // 4):
                pk = ps_main.tile([128, 1024], f32, tag="main")
                kr = krA if sl % 2 == 0 else krB
                for i in range(4):
                    t = 4 * sl + i
                    nc.tensor.matmul(pk[:, 256 * i:256 * (i + 1)],
                                     phi[:, 128 * t:128 * (t + 1)],
                                     pwphi[:, 256 * h:256 * (h + 1)],
                                     start=True, stop=False)
                    nc.tensor.matmul(pk[:, 256 * i:256 * (i + 1)],
                                     kfm[:, 128 * t:128 * (t + 1)],
                                     pwq[:, 256 * h:256 * (h + 1)],
                                     start=False, stop=True)
                for i in range(4):
                    t = 4 * sl + i
                    nc.scalar.activation(kr[:, i, 0:256],
                                         pk[:, 256 * i:256 * (i + 1)],
                                         AF.Exp, bias=negdgk[:, t:t + 1])
                    nc.vector.tensor_reduce(mxk[:, t:t + 1],
                                            pk[:, 256 * i:256 * (i + 1)],
                                            AX.X, OP.max)
                    nc.tensor.matmul(ctx_ps[:],
                                     vextm[:, t, 65 * h:65 * (h + 1)],
                                     kr[:, i, :],
                                     start=(t == 0), stop=(t == T - 1))

            # D. Q dash (feature-major) + raw exp
            qraw = [qraw_p.tile([128, NT], bf16, tag=f"qraw{c}",
                                name=f"qraw{c}") for c in range(2)]
            for cch in range(2):
                for dbl in range(NT // 1024):
                    pq = ps_main.tile([128, 1024], f32, tag="main")
                    for i in range(2):
                        off = 1024 * dbl + 512 * i
                        nc.tensor.matmul(
                            pq[:, 512 * i:512 * (i + 1)],
                            pwphi[:, 256 * h + 128 * cch:256 * h + 128 * (cch + 1)],
                            phi[:, off:off + 512],
                            start=True, stop=False)
                        nc.tensor.matmul(
                            pq[:, 512 * i:512 * (i + 1)],
                            pwq[:, 256 * h + 128 * cch:256 * h + 128 * (cch + 1)],
                            qfm[:, off:off + 512],
                            start=False, stop=True)
                    nc.scalar.activation(
                        qraw[cch][:, 1024 * dbl:1024 * (dbl + 1)], pq[:], AF.Exp)

            # E. m_glob, ctx fix-up, final lhsT assembly
            mkp = small.tile([128, 1], f32, tag="mkp")
            nc.vector.tensor_reduce(mkp[:], mxk[:], AX.X, OP.max)
            mkt_ps = ps_aux.tile([1, 128], f32, tag="aux")
            nc.tensor.transpose(mkt_ps[:], mkp[:], identf[:])
            mg = small.tile([1, 1], f32, tag="mg")
            nc.vector.tensor_reduce(mg[:], mkt_ps[:], AX.X, OP.max)
            emg = small.tile([1, 1], f32, tag="emg")
            nc.scalar.activation(emg[:], mg[:], AF.Exp, scale=-1.0)
            alpha11 = small.tile([1, 1], f32, tag="alpha11")
            nc.vector.tensor_scalar_mul(alpha11[:], emg[:], float(RATIO))
            alpha_col = small.tile([65, 1], f32, tag="alpha_col")
            nc.gpsimd.partition_broadcast(alpha_col[:], alpha11[:], channels=65)
            mveps = small.tile([65, 1], f32, tag="mveps")
            nc.vector.tensor_scalar_mul(mveps[:], ctx_ps[:, 256:257],
                                        float(RATIO * SOFTMAX_EPS))
            ctxT = small.tile([65, 256], f32, tag="ctxT")
            nc.vector.tensor_scalar(ctxT[:], ctx_ps[:, 0:256], alpha_col[:, 0:1],
                                    mveps[:, 0:1], OP.mult, OP.add)
            ctxTb = small.tile([65, 256], bf16, tag="ctxTb")
            nc.vector.tensor_copy(ctxTb[:], ctxT[:])

            ksrow = small.tile([1, 256], bf16, tag="ksrow")
            nc.sync.dma_start(ksrow[:], ctxTb[64:65, :])
            fin_lhsT = small.tile([128, 2, 65], bf16, tag="finl")
            for cch in range(2):
                c2_ps = ps_aux.tile([128, 64], f32, tag="aux")
                nc.tensor.matmul(c2_ps[:],
                                 ctxTb[0:64, 128 * cch:128 * (cch + 1)],
                                 ctxw[:, 64 * h:64 * (h + 1)],
                                 start=True, stop=True)
                nc.vector.tensor_copy(fin_lhsT[:, cch, 0:64], c2_ps[:])
                ks_ps = ps_aux.tile([128, 1], f32, tag="aux")
                nc.tensor.matmul(ks_ps[:],
                                 ksrow[0:1, 128 * cch:128 * (cch + 1)],
                                 onesb[0:1, 0:1], start=True, stop=True)
                nc.vector.tensor_copy(fin_lhsT[:, cch, 64:65], ks_ps[:])

            # F. exact exp(m_q): elementwise max of the two blocks, then
            # per-tile PE transpose + free-dim max
            qmc = qraw_p.tile([128, NT], bf16, tag="qmc")
            nc.vector.tensor_tensor(qmc[:], qraw[0][:], qraw[1][:], OP.max)
            rmax = small.tile([128, T], f32, tag="rmax")
            for t in range(T):
                pt = ps_aux.tile([128, 128], bf16, tag="aux")
                nc.tensor.transpose(pt[:], qmc[:, 128 * t:128 * (t + 1)],
                                    identb[:])
                nc.vector.tensor_reduce(rmax[:, t:t + 1], pt[:], AX.X, OP.max)

            # G. wcol = (NORM_EPS/ratio) * exp(dg_q) * rmax
            edq = small.tile([128, T], f32, tag="edq")
            nc.scalar.activation(edq[:], dgq[:, h, :], AF.Exp,
                                 bias=dgc8[:, h:h + 1],
                                 scale=float(0.5 * SOFTMAX_TEMP))
            wcol = small.tile([128, T], f32, tag="wcol")
            nc.vector.scalar_tensor_tensor(wcol[:], edq[:],
                                           float(NORM_EPS / RATIO),
                                           rmax[:], OP.mult, OP.mult)

            # H. final matmuls + normalize + accumulate
            for t in range(T):
                pf = ps_aux.tile([128, 65], f32, tag="aux")
                nc.tensor.matmul(pf[:], qraw[0][:, 128 * t:128 * (t + 1)],
                                 fin_lhsT[:, 0, :], start=True, stop=False)
                nc.tensor.matmul(pf[:], qraw[1][:, 128 * t:128 * (t + 1)],
                                 fin_lhsT[:, 1, :], start=False, stop=True)
                den = small.tile([128, 1], f32, tag="den")
                nc.vector.tensor_tensor(den[:], pf[:, 64:65], wcol[:, t:t + 1],
                                        OP.add)
                dinv = small.tile([128, 1], f32, tag="dinv")
                nc.vector.reciprocal(dinv[:], den[:])
                nc.vector.scalar_tensor_tensor(
                    acc[:, 64 * t:64 * (t + 1)], pf[:, 0:64], dinv[:, 0:1],
                    acc[:, 64 * t:64 * (t + 1)], OP.mult, OP.add)

        nc.sync.dma_start(
            outp_d.rearrange("(t p) e -> p t e", p=128),
            acc[:].rearrange("p (t e) -> p t e", e=64))

    return nc


def _get_nc(n_tok):
    if n_tok not in _CACHED:
        nc = build_nc(n_tok)
        nc.finalize()
        _CACHED[n_tok] = nc
    return _CACHED[n_tok]


_RUNNER = {}


def _get_runner(n_tok):
    """Cached jitted SPMD executor: in_maps(list of 8 dicts) -> list of outp."""
    if n_tok in _RUNNER:
        return _RUNNER[n_tok]
    import jax
    from jax.sharding import Mesh, PartitionSpec
    from jax.experimental.shard_map import shard_map
    from concourse import bass2jax
    from concourse.bass2jax import _bass_exec_p, partition_id_tensor

    bass2jax.install_neuronx_cc_hook()
    nc = _get_nc(n_tok)

    partition_name = (nc.partition_id_tensor.name
                      if nc.partition_id_tensor else None)
    in_names, out_names, out_avals, zero_outs = [], [], [], []
    for alloc in nc.m.functions[0].allocations:
        if not isinstance(alloc, mybir.MemoryLocationSet):
            continue
        name = alloc.memorylocations[0].name
        if alloc.kind == "ExternalInput":
            if name != partition_name:
                in_names.append(name)
        elif alloc.kind == "ExternalOutput":
            shape = tuple(alloc.tensor_shape)
            dtype = mybir.dt.np(alloc.dtype)
            out_names.append(name)
            out_avals.append(jax.core.ShapedArray(shape, dtype))
            zero_outs.append(np.zeros(shape, dtype))
    n_params = len(in_names)
    all_in_names = list(in_names) + list(out_names)
    if partition_name is not None:
        all_in_names.append(partition_name)
    donate = tuple(range(n_params, n_params + len(out_names)))

    def _body(*args):
        operands = list(args)
        if partition_name is not None:
            operands.append(partition_id_tensor())
        return tuple(_bass_exec_p.bind(
            *operands,
            out_avals=tuple(out_avals),
            in_names=tuple(all_in_names),
            out_names=tuple(out_names),
            lowering_input_output_aliases=(),
            sim_require_finite=True,
            sim_require_nnan=True,
            nc=nc,
        ))

    devices = jax.devices()[:8]
    mesh = Mesh(np.asarray(devices), ("core",))
    nio = n_params + len(out_names)
    sharded = jax.jit(
        shard_map(_body, mesh=mesh,
                  in_specs=(PartitionSpec("core"),) * nio,
                  out_specs=(PartitionSpec("core"),) * len(out_names),
                  check_rep=False),
        donate_argnums=donate, keep_unused=True)

    def run(in_maps, reps=1, device_inputs=None):
        if device_inputs is None:
            concat_in = [np.concatenate([in_maps[c][n] for c in range(8)], axis=0)
                         for n in in_names]
        else:
            concat_in = device_inputs
        outs = None
        for _ in range(reps):
            zeros = [np.zeros((8 * z.shape[0], *z.shape[1:]), z.dtype)
                     for z in zero_outs]
            outs = sharded(*concat_in, *zeros)
        arrs = [np.asarray(o) for o in outs]
        return [
            {name: arrs[i].reshape(8, *out_avals[i].shape)[c]
             for i, name in enumerate(out_names)}
            for c in range(8)
        ]

    def put(in_maps):
        from jax.sharding import NamedSharding
        sh = NamedSharding(mesh, PartitionSpec("core"))
        return [jax.device_put(
            np.concatenate([in_maps[c][n] for c in range(8)], axis=0), sh)
            for n in in_names]

    run.put = put

    _RUNNER[n_tok] = run
    return run


def kernel(**inputs):
    n_tok = N_TOK
    in_maps = host_prep(n_tok=n_tok, **inputs)
    run = _get_runner(n_tok)
    results = run(in_maps)
    out_b = np.asarray(inputs["out_b"])
    n = inputs["query"].shape[1]
    out = np.zeros((B, n, DH), np.float32)
    for b in range(B):
        part = results[2 * b]["outp"] + results[2 * b + 1]["outp"]
        out[b, :n_tok] = part + out_b[None, :]
    return out
